# revision 33
# baseline (speedup 1.0000x reference)
"""CPC criterion (contrastive predictive coding loss) on 8 TRN2 NeuronCores.

Strategy (data-parallel over batch, per sharding hint):
  - Core b handles batch element b (B == 8 == n_cores).
  - The per-row gather of K=128 negatives has no efficient per-partition
    gather primitive on TRN2, so it is reformulated as a dense masked
    reduction:  S[t] = sum_m cnt[t, m] * exp(q_t * L[t, m])
    where L = zh_unnorm @ pool_norm.T is the full-pool logit matrix
    (a PE-friendly matmul), q_t = 1/(|zh_t| * temperature), and
    cnt[t, m] (built on the host from neg_idx) counts how many times pool
    entry m appears among row t's negatives, +1 at the positive target.
    Then lse[t] = log S[t], and loss rows are lse[t] - pos_logit[t].
  - Per-core device output: per-horizon partial sums [sum_t log S, sum_t pos];
    host combines the 8 cores (the scalar "all-reduce") and applies the
    1/(Nk) and 1/n_pred factors.

Device pipeline per horizon k (Tk = 255-kk rows on this core):
  zhT[e,t]   = sum_c WT[k][c,e] * ctxT[c,t]          (PE, psum f32 -> bf16 sbuf)
  sumsq/pos  = ones-matmul partition reductions of zh^2 and zh*encbT_n
               (muls on GpSimd, reduction on PE)
  q          = exp(-0.5*Ln(ss) + ln(1/temp))         (ACT only — keeps the
               Exp/Ln/Copy table set loaded; no Sqrt set switch)
  L          = zhT.T @ poolT_n                       (PE, 4 x 512-col chunks)
  E          = exp(q * L)                            (ACT, psum -> bf16 sbuf)
  S          = sum_m E * cnt                         (DVE affine_mul_reduce)
  logS       = Ln(S); sum_t via ones-matmul          (ACT + PE)
"""

import math

import numpy as np
import ml_dtypes

import concourse.bass as bass
import concourse.tile as tile
import concourse.mybir as mybir
from concourse import bacc
from concourse import bass_utils

B, T, C_CTX, C_ENC, NPRED, K_NEG = 8, 256, 512, 256, 12, 128
POOL = B * T  # 2048
TEMP = 0.07
N_CORES = 8

DT = mybir.dt
BF16 = ml_dtypes.bfloat16
FP8 = ml_dtypes.float8_e4m3

_CACHE = {}


class _OneActSetBacc(bacc.Bacc):
    """Bacc whose act-table pass sees only the natural_log_exp_and_others
    set (it contains Exp, Ln, Copy, Identity, Square — everything this
    kernel uses), so exactly one ACT_TABLE_LOAD is emitted instead of a
    reload on every Exp<->Ln<->Copy transition (1.3us each). Set indices
    are preserved; the other sets are just emptied so they are never
    chosen."""

    def insert_act_table_loads(self):
        import bass_rust as _bass_rust
        from concourse.hw_specs import get_activation_tables

        has_activation = any(
            isinstance(i, mybir.InstActivation)
            for b in self.main_func.blocks
            for i in b.instructions
        )
        if not has_activation:
            return
        keep = "natural_log_exp_and_others"
        tables = [
            (name, funcs if name == keep else set())
            for name, funcs in get_activation_tables(self.m.arch).items()
        ]
        _bass_rust.insert_act_table_loads(self, tables)


def _build_program():
    nc = _OneActSetBacc(
        "TRN2",
        target_bir_lowering=False,
        debug=False,
        enable_asserts=False,
        num_devices=N_CORES,
    )
    ctxT = nc.dram_tensor("ctxT", [C_CTX, T], DT.bfloat16, kind="ExternalInput").ap()
    WT = nc.dram_tensor("WT", [NPRED, C_CTX, C_ENC], DT.bfloat16, kind="ExternalInput").ap()
    poolT = nc.dram_tensor("poolT", [C_ENC, POOL], DT.bfloat16, kind="ExternalInput").ap()
    encbT = nc.dram_tensor("encbT", [C_ENC, T], DT.bfloat16, kind="ExternalInput").ap()
    cnt = nc.dram_tensor("cnt", [NPRED, T, POOL], DT.float8e4, kind="ExternalInput").ap()
    out_sums = nc.dram_tensor("out_sums", [1, 2 * NPRED], DT.float32, kind="ExternalOutput").ap()

    with tile.TileContext(nc) as tc:
        _emit(nc, tc, ctxT, WT, poolT, encbT, cnt, out_sums)
    nc.compile()
    return nc


def _emit(nc, tc, ctxT, WT, poolT, encbT, cnt, out_sums):
    import contextlib

    f32 = DT.float32
    bf16 = DT.bfloat16
    AF = mybir.ActivationFunctionType
    ALU = mybir.AluOpType

    ctx = contextlib.ExitStack()
    with ctx:
        singles = ctx.enter_context(tc.tile_pool(name="singles", bufs=1))
        wt_pool = ctx.enter_context(tc.tile_pool(name="wt", bufs=3))
        zh_pool = ctx.enter_context(tc.tile_pool(name="zh", bufs=2))
        big_pool = ctx.enter_context(tc.tile_pool(name="big", bufs=3))
        e_pool = ctx.enter_context(tc.tile_pool(name="epool", bufs=3))
        cnt_pool = ctx.enter_context(tc.tile_pool(name="cntp", bufs=4))
        tout_pool = ctx.enter_context(tc.tile_pool(name="tout", bufs=3))
        sv_pool = ctx.enter_context(tc.tile_pool(name="sv", bufs=8))
        qcol_pool = ctx.enter_context(tc.tile_pool(name="qcol", bufs=2))
        rows_pool = ctx.enter_context(tc.tile_pool(name="rows", bufs=3))
        psum_L = ctx.enter_context(tc.tile_pool(name="psL", bufs=3, space="PSUM"))
        psum_zh = ctx.enter_context(tc.tile_pool(name="psz", bufs=1, space="PSUM"))
        psum_sp = ctx.enter_context(tc.tile_pool(name="pssp", bufs=1, space="PSUM"))
        dram = ctx.enter_context(tc.tile_pool(name="dram", bufs=1, space="DRAM"))

        # ---- one-time loads ----
        ctx_sb = []
        for cc in range(4):
            t_ = singles.tile([128, T], bf16, tag=f"ctx{cc}")
            nc.sync.dma_start(out=t_, in_=ctxT[cc * 128:(cc + 1) * 128, :])
            ctx_sb.append(t_)
        pool_sb = []
        encb_sb = []
        for eh in range(2):
            p_ = singles.tile([128, POOL], bf16, tag=f"pool{eh}")
            nc.gpsimd.dma_start(out=p_, in_=poolT[eh * 128:(eh + 1) * 128, :])
            pool_sb.append(p_)
            e_ = singles.tile([128, T], bf16, tag=f"encb{eh}")
            nc.gpsimd.dma_start(out=e_, in_=encbT[eh * 128:(eh + 1) * 128, :])
            encb_sb.append(e_)

        ones_bf = singles.tile([128, 1], bf16, tag="ones_bf")
        nc.vector.memset(ones_bf, 1.0)
        ones_f32 = singles.tile([128, 1], f32, tag="ones_f32")
        nc.vector.memset(ones_f32, 1.0)
        outbuf = singles.tile([1, 2 * NPRED], f32, tag="outbuf")
        coll = singles.tile([128, 4 * NPRED], f32, tag="coll")
        nc.vector.memset(coll, 0.0)
        warm_rhs = singles.tile([128, 512], bf16, tag="warm_rhs")
        nc.vector.memset(warm_rhs, 0.0)
        biasq = singles.tile([128, 1], f32, tag="biasq")
        nc.vector.memset(biasq, float(math.log(1.0 / TEMP)))


        # ---- phase 1: all horizons' zh, norms, pos sums, and q columns ----
        zh_all = {}
        qcol_all = {}
        # warm the PE clock gate (HAM) with throwaway matmuls while the
        # input DMAs are in flight; ~4us of PE activity flips K to 8/8.
        wps = psum_zh.tile([128, 2, 255], f32, tag="zhps", name="warm_ps")
        for i in range(10):
            nc.tensor.matmul(wps[:1, 0, :128], ones_bf, warm_rhs[:, :128],
                             start=(i == 0), stop=(i == 9))

        def emit_p1(kk):
            Tk = T - 1 - kk  # 255 - kk rows on this core
            R = [128, Tk - 128]

            # W[k].T in one DMA: [c(4x128), e 256] -> [128, 4, 256]
            wtile = wt_pool.tile([128, 4, C_ENC], bf16, tag="wt")
            nc.sync.dma_start(
                out=wtile,
                in_=WT[kk].rearrange("(cc p) e -> p cc e", p=128),
            )

            # matmul1: zhT[e, t] (2 e-halves in one psum bank)
            zps = psum_zh.tile([128, 2, Tk], f32, tag="zhps")
            for eh in range(2):
                for cc in range(4):
                    nc.tensor.matmul(
                        zps[:, eh, :],
                        wtile[:, cc, eh * 128:(eh + 1) * 128],
                        ctx_sb[cc][:, :Tk],
                        start=(cc == 0),
                        stop=(cc == 3),
                    )
            z_ = zh_pool.tile([128, 2, Tk], bf16, tag=f"zh{kk}", name=f"zh{kk}")
            nc.scalar.activation(out=z_, in_=zps, func=AF.Copy)
            zh_sb = [z_[:, 0, :], z_[:, 1, :]]
            zh_all[kk] = zh_sb

            # zh^2 and zh*pos packed into one [128, 2*Tk] tile per e-half
            bgs = []
            for eh in range(2):
                bg = big_pool.tile([128, 2 * Tk], bf16, tag=f"big{eh}")
                nc.gpsimd.tensor_mul(bg[:, :Tk], zh_sb[eh], zh_sb[eh])
                nc.gpsimd.tensor_mul(
                    bg[:, Tk:2 * Tk], zh_sb[eh], encb_sb[eh][:, kk + 1:kk + 1 + Tk]
                )
                bgs.append(bg)

            # per t-half: ss and pos column vectors via tiny ones-rhs matmuls
            # (bg as lhsT: out[t, 1] = sum_e bg[e, t]); then
            # q = 1/(sqrt(ss)*TEMP) = exp(-0.5*ln(ss) + ln(1/TEMP)) on columns.
            q_col = []
            for h in range(2):
                Rh = R[h]
                sp2 = psum_sp.tile([128, 2], f32, tag="sp", name=f"sp{kk}_{h}")
                for eh in range(2):
                    nc.tensor.matmul(
                        sp2[:Rh, 0:1],
                        bgs[eh][:, h * 128:h * 128 + Rh],
                        ones_bf,
                        start=(eh == 0),
                        stop=(eh == 1),
                    )
                for eh in range(2):
                    nc.tensor.matmul(
                        sp2[:Rh, 1:2],
                        bgs[eh][:, Tk + h * 128:Tk + h * 128 + Rh],
                        ones_bf,
                        start=(eh == 0),
                        stop=(eh == 1),
                    )
                lnc = sv_pool.tile([128, 1], f32, tag="lnc")
                nc.scalar.activation(out=lnc[:Rh, :], in_=sp2[:Rh, 0:1], func=AF.Ln)
                qc = qcol_pool.tile([128, 1], f32, tag=f"qc{kk}_{h}", name=f"qc{kk}_{h}")
                nc.scalar.activation(
                    out=qc[:Rh, :], in_=lnc[:Rh, :], func=AF.Exp,
                    scale=-0.5, bias=biasq[:Rh, :],
                )
                # pos~ column straight into the collection tile (cols 24..47)
                nc.vector.tensor_mul(
                    coll[:Rh, 2 * NPRED + 2 * kk + h:2 * NPRED + 2 * kk + h + 1],
                    sp2[:Rh, 1:2],
                    qc[:Rh, :],
                )
                q_col.append(qc[:Rh, :])
            qcol_all[kk] = q_col

        # ---- phase 2: full-pool logits, exp, masked reduce per horizon ----
        def emit_p2(kk):
            Tk = T - 1 - kk
            R = [128, Tk - 128]
            zh_sb = zh_all[kk]
            q_col = qcol_all[kk]

            for h in range(2):
                Rh = R[h]
                cb = cnt_pool.tile([128, POOL], DT.float8e4, tag="cnt")
                nc.sync.dma_start(
                    out=cb[:Rh, :],
                    in_=cnt[kk, h * 128:h * 128 + Rh, :],
                )
                ee = e_pool.tile([128, POOL], bf16, tag="E")
                for g in range(2):
                    lp = psum_L.tile([128, 2, 512], f32, tag="L", name=f"lp{g}")
                    for i in range(2):
                        mc = 2 * g + i
                        for eh in range(2):
                            nc.tensor.matmul(
                                lp[:Rh, i, :],
                                zh_sb[eh][:, h * 128:h * 128 + Rh],
                                pool_sb[eh][:, mc * 512:(mc + 1) * 512],
                                start=(eh == 0),
                                stop=(eh == 1),
                            )
                    nc.scalar.activation(
                        out=ee[:Rh, g * 1024:(g + 1) * 1024],
                        in_=lp[:Rh, :, :],
                        func=AF.Exp,
                        scale=q_col[h],
                    )
                to_ = tout_pool.tile([128, POOL], bf16, tag="to")
                s_cur = sv_pool.tile([128, 1], f32, tag="scur")
                nc.vector.affine_mul_reduce(
                    out=to_[:Rh, :],
                    accum_out=s_cur[:Rh, :],
                    in0=ee[:Rh, :],
                    in1=cb[:Rh, :],
                    scale=1.0,
                    bias=0.0,
                )
                j = 2 * kk + h
                nc.scalar.activation(
                    out=coll[:Rh, j:j + 1], in_=s_cur[:Rh, :], func=AF.Ln
                )

        # interleaved drive: phase 1 stays two horizons ahead of phase 2
        emit_p1(0)
        emit_p1(1)
        for kk in range(NPRED):
            emit_p2(kk)
            if kk + 2 < NPRED:
                emit_p1(kk + 2)

        fin = psum_sp.tile([1, 4 * NPRED], f32, tag="sp", name="fin")
        nc.tensor.matmul(fin, ones_f32, coll, start=True, stop=True)
        fin_sb = singles.tile([1, 4 * NPRED], f32, tag="fin_sb")
        nc.vector.tensor_copy(out=fin_sb, in_=fin)
        nc.vector.tensor_add(
            outbuf[:, 0:2 * NPRED],
            fin_sb[:, 0:4 * NPRED:2],
            fin_sb[:, 1:4 * NPRED:2],
        )
        nc.sync.dma_start(out=out_sums, in_=outbuf)


def _build_cnt(neg_idx):
    """cnt[b, kk, t, m] (uint8): negative multiplicities + 1 at the positive."""
    cnt = np.zeros((B, NPRED, T, POOL), dtype=np.uint8)
    b_ar = np.arange(B, dtype=np.int64)[:, None]
    for kk in range(NPRED):
        Tk = T - 1 - kk
        Nk = B * Tk
        idx = np.asarray(neg_idx[kk, :Nk], dtype=np.int64)  # [Nk, K]
        flat = idx + np.arange(Nk, dtype=np.int64)[:, None] * POOL
        c = np.bincount(flat.ravel(), minlength=Nk * POOL)
        c = c.astype(np.uint8).reshape(B, Tk, POOL)
        t_ar = np.arange(Tk, dtype=np.int64)[None, :]
        pos_m = T * b_ar + (kk + 1) + t_ar
        c[b_ar, t_ar, pos_m] += 1
        cnt[:, kk, :Tk, :] = c
    return cnt


def kernel(context, encoded, W, neg_idx):
    context = np.asarray(context, dtype=np.float32)
    encoded = np.asarray(encoded, dtype=np.float32)
    W = np.asarray(W, dtype=np.float32)
    neg_idx = np.asarray(neg_idx)

    if "nc" not in _CACHE:
        _CACHE["nc"] = _build_program()
    nc = _CACHE["nc"]

    # host prep: layout transposes, normalized pool, count mask
    pool = encoded.reshape(POOL, C_ENC)
    pool_n = pool / np.linalg.norm(pool, axis=-1, keepdims=True)
    poolT_bf = np.ascontiguousarray(pool_n.T).astype(BF16)  # [256, 2048]
    WT_bf = np.ascontiguousarray(W.transpose(0, 2, 1)).astype(BF16)  # [12, 512, 256]
    cnt_u8 = _build_cnt(neg_idx)
    fp8_lut = np.arange(256).astype(np.float32).astype(FP8)

    in_maps = []
    for b in range(N_CORES):
        in_maps.append(
            {
                "ctxT": np.ascontiguousarray(context[b].T).astype(BF16),
                "WT": WT_bf,
                "poolT": poolT_bf,
                "encbT": np.ascontiguousarray(poolT_bf[:, T * b:T * (b + 1)]),
                "cnt": fp8_lut[cnt_u8[b]],
            }
        )

    _CACHE["in_maps"] = in_maps
    res = bass_utils.run_bass_kernel_spmd(nc, in_maps, core_ids=list(range(N_CORES)))

    total = np.float64(0.0)
    for kk in range(NPRED):
        Tk = T - 1 - kk
        num = np.float64(0.0)
        for b in range(N_CORES):
            sums = res.results[b]["out_sums"][0]
            num += np.float64(sums[kk]) - np.float64(sums[NPRED + kk])
        total += num / (B * Tk)
    total = total / NPRED
    return np.float32(total)


# revision 34
# speedup vs baseline: 1.0469x; 1.0469x over previous
"""CPC criterion (contrastive predictive coding loss) on 8 TRN2 NeuronCores.

Strategy (data-parallel over batch, per sharding hint):
  - Core b handles batch element b (B == 8 == n_cores).
  - The per-row gather of K=128 negatives has no efficient per-partition
    gather primitive on TRN2, so it is reformulated as a dense masked
    reduction:  S[t] = sum_m cnt[t, m] * exp(q_t * L[t, m])
    where L = zh_unnorm @ pool_norm.T is the full-pool logit matrix
    (a PE-friendly matmul), q_t = 1/(|zh_t| * temperature), and
    cnt[t, m] (built on the host from neg_idx) counts how many times pool
    entry m appears among row t's negatives, +1 at the positive target.
    Then lse[t] = log S[t], and loss rows are lse[t] - pos_logit[t].
  - Per-core device output: per-horizon partial sums [sum_t log S, sum_t pos];
    host combines the 8 cores (the scalar "all-reduce") and applies the
    1/(Nk) and 1/n_pred factors.

Device pipeline per horizon k (Tk = 255-kk rows on this core):
  zhT[e,t]   = sum_c WT[k][c,e] * ctxT[c,t]          (PE, psum f32 -> bf16 sbuf)
  sumsq/pos  = ones-matmul partition reductions of zh^2 and zh*encbT_n
               (muls on GpSimd, reduction on PE)
  q          = exp(-0.5*Ln(ss) + ln(1/temp))         (ACT only — keeps the
               Exp/Ln/Copy table set loaded; no Sqrt set switch)
  L          = zhT.T @ poolT_n                       (PE, 4 x 512-col chunks)
  E          = exp(q * L)                            (ACT, psum -> bf16 sbuf)
  S          = sum_m E * cnt                         (DVE affine_mul_reduce)
  logS       = Ln(S); sum_t via ones-matmul          (ACT + PE)
"""

import math

import numpy as np
import ml_dtypes

import concourse.bass as bass
import concourse.tile as tile
import concourse.mybir as mybir
from concourse import bacc
from concourse import bass_utils

B, T, C_CTX, C_ENC, NPRED, K_NEG = 8, 256, 512, 256, 12, 128
POOL = B * T  # 2048
TEMP = 0.07
N_CORES = 8

DT = mybir.dt
BF16 = ml_dtypes.bfloat16
FP8 = ml_dtypes.float8_e4m3

_CACHE = {}


class _OneActSetBacc(bacc.Bacc):
    """Bacc whose act-table pass sees only the natural_log_exp_and_others
    set (it contains Exp, Ln, Copy, Identity, Square — everything this
    kernel uses), so exactly one ACT_TABLE_LOAD is emitted instead of a
    reload on every Exp<->Ln<->Copy transition (1.3us each). Set indices
    are preserved; the other sets are just emptied so they are never
    chosen."""

    def insert_act_table_loads(self):
        import bass_rust as _bass_rust
        from concourse.hw_specs import get_activation_tables

        has_activation = any(
            isinstance(i, mybir.InstActivation)
            for b in self.main_func.blocks
            for i in b.instructions
        )
        if not has_activation:
            return
        keep = "natural_log_exp_and_others"
        tables = [
            (name, funcs if name == keep else set())
            for name, funcs in get_activation_tables(self.m.arch).items()
        ]
        _bass_rust.insert_act_table_loads(self, tables)


def _build_program():
    nc = _OneActSetBacc(
        "TRN2",
        target_bir_lowering=False,
        debug=False,
        enable_asserts=False,
        num_devices=N_CORES,
    )
    ctxT = nc.dram_tensor("ctxT", [C_CTX, T], DT.bfloat16, kind="ExternalInput").ap()
    WT = nc.dram_tensor("WT", [NPRED, C_CTX, C_ENC], DT.bfloat16, kind="ExternalInput").ap()
    poolT = nc.dram_tensor("poolT", [C_ENC, POOL], DT.bfloat16, kind="ExternalInput").ap()
    encbT = nc.dram_tensor("encbT", [C_ENC, T], DT.bfloat16, kind="ExternalInput").ap()
    cnt = nc.dram_tensor("cnt", [NPRED, T, POOL], DT.float8e4, kind="ExternalInput").ap()
    out_sums = nc.dram_tensor("out_sums", [1, 2 * NPRED], DT.float32, kind="ExternalOutput").ap()

    with tile.TileContext(nc) as tc:
        _emit(nc, tc, ctxT, WT, poolT, encbT, cnt, out_sums)
    nc.compile()
    return nc


def _emit(nc, tc, ctxT, WT, poolT, encbT, cnt, out_sums):
    import contextlib

    f32 = DT.float32
    bf16 = DT.bfloat16
    AF = mybir.ActivationFunctionType
    ALU = mybir.AluOpType

    ctx = contextlib.ExitStack()
    with ctx:
        singles = ctx.enter_context(tc.tile_pool(name="singles", bufs=1))
        wt_pool = ctx.enter_context(tc.tile_pool(name="wt", bufs=3))
        zh_pool = ctx.enter_context(tc.tile_pool(name="zh", bufs=2))
        big_pool = ctx.enter_context(tc.tile_pool(name="big", bufs=3))
        e_pool = ctx.enter_context(tc.tile_pool(name="epool", bufs=3))
        cnt_pool = ctx.enter_context(tc.tile_pool(name="cntp", bufs=4))
        tout_pool = ctx.enter_context(tc.tile_pool(name="tout", bufs=3))
        sv_pool = ctx.enter_context(tc.tile_pool(name="sv", bufs=8))
        qcol_pool = ctx.enter_context(tc.tile_pool(name="qcol", bufs=2))
        rows_pool = ctx.enter_context(tc.tile_pool(name="rows", bufs=3))
        psum_L = ctx.enter_context(tc.tile_pool(name="psL", bufs=3, space="PSUM"))
        psum_zh = ctx.enter_context(tc.tile_pool(name="psz", bufs=1, space="PSUM"))
        psum_sp = ctx.enter_context(tc.tile_pool(name="pssp", bufs=1, space="PSUM"))
        dram = ctx.enter_context(tc.tile_pool(name="dram", bufs=1, space="DRAM"))

        # ---- one-time loads ----
        ctx_sb = []
        for cc in range(4):
            t_ = singles.tile([128, T], bf16, tag=f"ctx{cc}")
            nc.sync.dma_start(out=t_, in_=ctxT[cc * 128:(cc + 1) * 128, :])
            ctx_sb.append(t_)
        pool_sb = []
        encb_sb = []
        for eh in range(2):
            p_ = singles.tile([128, POOL], bf16, tag=f"pool{eh}")
            nc.sync.dma_start(out=p_, in_=poolT[eh * 128:(eh + 1) * 128, :])
            pool_sb.append(p_)
            e_ = singles.tile([128, T], bf16, tag=f"encb{eh}")
            nc.sync.dma_start(out=e_, in_=encbT[eh * 128:(eh + 1) * 128, :])
            encb_sb.append(e_)

        ones_bf = singles.tile([128, 1], bf16, tag="ones_bf")
        nc.vector.memset(ones_bf, 1.0)
        ones_f32 = singles.tile([128, 1], f32, tag="ones_f32")
        nc.vector.memset(ones_f32, 1.0)
        outbuf = singles.tile([1, 2 * NPRED], f32, tag="outbuf")
        coll = singles.tile([128, 4 * NPRED], f32, tag="coll")
        nc.vector.memset(coll, 0.0)
        warm_rhs = singles.tile([128, 512], bf16, tag="warm_rhs")
        nc.vector.memset(warm_rhs, 0.0)
        biasq = singles.tile([128, 1], f32, tag="biasq")
        nc.vector.memset(biasq, float(math.log(1.0 / TEMP)))


        # ---- phase 1: all horizons' zh, norms, pos sums, and q columns ----
        zh_all = {}
        qcol_all = {}
        # warm the PE clock gate (HAM) with throwaway matmuls while the
        # input DMAs are in flight; ~4us of PE activity flips K to 8/8.
        wps = psum_zh.tile([128, 2, 255], f32, tag="zhps", name="warm_ps")
        for i in range(10):
            nc.tensor.matmul(wps[:1, 0, :255], ones_bf, warm_rhs[:, :255],
                             start=(i == 0), stop=(i == 9))

        def emit_p1(kk):
            Tk = T - 1 - kk  # 255 - kk rows on this core
            R = [128, Tk - 128]

            # W[k].T in one DMA: [c(4x128), e 256] -> [128, 4, 256]
            wtile = wt_pool.tile([128, 4, C_ENC], bf16, tag="wt")
            nc.sync.dma_start(
                out=wtile,
                in_=WT[kk].rearrange("(cc p) e -> p cc e", p=128),
            )

            # matmul1: zhT[e, t] (2 e-halves in one psum bank)
            zps = psum_zh.tile([128, 2, Tk], f32, tag="zhps")
            for eh in range(2):
                for cc in range(4):
                    nc.tensor.matmul(
                        zps[:, eh, :],
                        wtile[:, cc, eh * 128:(eh + 1) * 128],
                        ctx_sb[cc][:, :Tk],
                        start=(cc == 0),
                        stop=(cc == 3),
                    )
            z_ = zh_pool.tile([128, 2, Tk], bf16, tag=f"zh{kk}", name=f"zh{kk}")
            nc.scalar.activation(out=z_, in_=zps, func=AF.Copy)
            zh_sb = [z_[:, 0, :], z_[:, 1, :]]
            zh_all[kk] = zh_sb

            # zh^2 and zh*pos packed into one [128, 2*Tk] tile per e-half
            bgs = []
            for eh in range(2):
                bg = big_pool.tile([128, 2 * Tk], bf16, tag=f"big{eh}")
                nc.gpsimd.tensor_mul(bg[:, :Tk], zh_sb[eh], zh_sb[eh])
                nc.gpsimd.tensor_mul(
                    bg[:, Tk:2 * Tk], zh_sb[eh], encb_sb[eh][:, kk + 1:kk + 1 + Tk]
                )
                bgs.append(bg)

            # per t-half: ss and pos column vectors via tiny ones-rhs matmuls
            # (bg as lhsT: out[t, 1] = sum_e bg[e, t]); then
            # q = 1/(sqrt(ss)*TEMP) = exp(-0.5*ln(ss) + ln(1/TEMP)) on columns.
            q_col = []
            for h in range(2):
                Rh = R[h]
                sp2 = psum_sp.tile([128, 2], f32, tag="sp", name=f"sp{kk}_{h}")
                for eh in range(2):
                    nc.tensor.matmul(
                        sp2[:Rh, 0:1],
                        bgs[eh][:, h * 128:h * 128 + Rh],
                        ones_bf,
                        start=(eh == 0),
                        stop=(eh == 1),
                    )
                for eh in range(2):
                    nc.tensor.matmul(
                        sp2[:Rh, 1:2],
                        bgs[eh][:, Tk + h * 128:Tk + h * 128 + Rh],
                        ones_bf,
                        start=(eh == 0),
                        stop=(eh == 1),
                    )
                lnc = sv_pool.tile([128, 1], f32, tag="lnc")
                nc.scalar.activation(out=lnc[:Rh, :], in_=sp2[:Rh, 0:1], func=AF.Ln)
                qc = qcol_pool.tile([128, 1], f32, tag=f"qc{kk}_{h}", name=f"qc{kk}_{h}")
                nc.scalar.activation(
                    out=qc[:Rh, :], in_=lnc[:Rh, :], func=AF.Exp,
                    scale=-0.5, bias=biasq[:Rh, :],
                )
                # pos~ column straight into the collection tile (cols 24..47)
                nc.vector.tensor_mul(
                    coll[:Rh, 2 * NPRED + 2 * kk + h:2 * NPRED + 2 * kk + h + 1],
                    sp2[:Rh, 1:2],
                    qc[:Rh, :],
                )
                q_col.append(qc[:Rh, :])
            qcol_all[kk] = q_col

        # ---- phase 2: full-pool logits, exp, masked reduce per horizon ----
        def emit_p2(kk):
            Tk = T - 1 - kk
            R = [128, Tk - 128]
            zh_sb = zh_all[kk]
            q_col = qcol_all[kk]

            for h in range(2):
                Rh = R[h]
                cb = cnt_pool.tile([128, POOL], DT.float8e4, tag="cnt")
                nc.sync.dma_start(
                    out=cb[:Rh, :],
                    in_=cnt[kk, h * 128:h * 128 + Rh, :],
                )
                ee = e_pool.tile([128, POOL], bf16, tag="E")
                for g in range(2):
                    lp = psum_L.tile([128, 2, 512], f32, tag="L", name=f"lp{g}")
                    for i in range(2):
                        mc = 2 * g + i
                        for eh in range(2):
                            nc.tensor.matmul(
                                lp[:Rh, i, :],
                                zh_sb[eh][:, h * 128:h * 128 + Rh],
                                pool_sb[eh][:, mc * 512:(mc + 1) * 512],
                                start=(eh == 0),
                                stop=(eh == 1),
                            )
                    nc.scalar.activation(
                        out=ee[:Rh, g * 1024:(g + 1) * 1024],
                        in_=lp[:Rh, :, :],
                        func=AF.Exp,
                        scale=q_col[h],
                    )
                to_ = tout_pool.tile([128, POOL], bf16, tag="to")
                s_cur = sv_pool.tile([128, 1], f32, tag="scur")
                nc.vector.affine_mul_reduce(
                    out=to_[:Rh, :],
                    accum_out=s_cur[:Rh, :],
                    in0=ee[:Rh, :],
                    in1=cb[:Rh, :],
                    scale=1.0,
                    bias=0.0,
                )
                j = 2 * kk + h
                nc.scalar.activation(
                    out=coll[:Rh, j:j + 1], in_=s_cur[:Rh, :], func=AF.Ln
                )

        for kk in range(NPRED):
            emit_p1(kk)
        for kk in range(NPRED):
            emit_p2(kk)

        fin = psum_sp.tile([1, 4 * NPRED], f32, tag="sp", name="fin")
        nc.tensor.matmul(fin, ones_f32, coll, start=True, stop=True)
        fin_sb = singles.tile([1, 4 * NPRED], f32, tag="fin_sb")
        nc.vector.tensor_copy(out=fin_sb, in_=fin)
        nc.vector.tensor_add(
            outbuf[:, 0:2 * NPRED],
            fin_sb[:, 0:4 * NPRED:2],
            fin_sb[:, 1:4 * NPRED:2],
        )
        nc.sync.dma_start(out=out_sums, in_=outbuf)


def _build_cnt(neg_idx):
    """cnt[b, kk, t, m] (uint8): negative multiplicities + 1 at the positive."""
    cnt = np.zeros((B, NPRED, T, POOL), dtype=np.uint8)
    b_ar = np.arange(B, dtype=np.int64)[:, None]
    for kk in range(NPRED):
        Tk = T - 1 - kk
        Nk = B * Tk
        idx = np.asarray(neg_idx[kk, :Nk], dtype=np.int64)  # [Nk, K]
        flat = idx + np.arange(Nk, dtype=np.int64)[:, None] * POOL
        c = np.bincount(flat.ravel(), minlength=Nk * POOL)
        c = c.astype(np.uint8).reshape(B, Tk, POOL)
        t_ar = np.arange(Tk, dtype=np.int64)[None, :]
        pos_m = T * b_ar + (kk + 1) + t_ar
        c[b_ar, t_ar, pos_m] += 1
        cnt[:, kk, :Tk, :] = c
    return cnt


def kernel(context, encoded, W, neg_idx):
    context = np.asarray(context, dtype=np.float32)
    encoded = np.asarray(encoded, dtype=np.float32)
    W = np.asarray(W, dtype=np.float32)
    neg_idx = np.asarray(neg_idx)

    if "nc" not in _CACHE:
        _CACHE["nc"] = _build_program()
    nc = _CACHE["nc"]

    # host prep: layout transposes, normalized pool, count mask
    pool = encoded.reshape(POOL, C_ENC)
    pool_n = pool / np.linalg.norm(pool, axis=-1, keepdims=True)
    poolT_bf = np.ascontiguousarray(pool_n.T).astype(BF16)  # [256, 2048]
    WT_bf = np.ascontiguousarray(W.transpose(0, 2, 1)).astype(BF16)  # [12, 512, 256]
    cnt_u8 = _build_cnt(neg_idx)
    fp8_lut = np.arange(256).astype(np.float32).astype(FP8)

    in_maps = []
    for b in range(N_CORES):
        in_maps.append(
            {
                "ctxT": np.ascontiguousarray(context[b].T).astype(BF16),
                "WT": WT_bf,
                "poolT": poolT_bf,
                "encbT": np.ascontiguousarray(poolT_bf[:, T * b:T * (b + 1)]),
                "cnt": fp8_lut[cnt_u8[b]],
            }
        )

    _CACHE["in_maps"] = in_maps
    res = bass_utils.run_bass_kernel_spmd(nc, in_maps, core_ids=list(range(N_CORES)))

    total = np.float64(0.0)
    for kk in range(NPRED):
        Tk = T - 1 - kk
        num = np.float64(0.0)
        for b in range(N_CORES):
            sums = res.results[b]["out_sums"][0]
            num += np.float64(sums[kk]) - np.float64(sums[NPRED + kk])
        total += num / (B * Tk)
    total = total / NPRED
    return np.float32(total)


# revision 35
# speedup vs baseline: 1.0669x; 1.0191x over previous
"""CPC criterion (contrastive predictive coding loss) on 8 TRN2 NeuronCores.

Strategy (data-parallel over batch, per sharding hint):
  - Core b handles batch element b (B == 8 == n_cores).
  - The per-row gather of K=128 negatives has no efficient per-partition
    gather primitive on TRN2, so it is reformulated as a dense masked
    reduction:  S[t] = sum_m cnt[t, m] * exp(q_t * L[t, m])
    where L = zh_unnorm @ pool_norm.T is the full-pool logit matrix
    (a PE-friendly matmul), q_t = 1/(|zh_t| * temperature), and
    cnt[t, m] (built on the host from neg_idx) counts how many times pool
    entry m appears among row t's negatives, +1 at the positive target.
    Then lse[t] = log S[t], and loss rows are lse[t] - pos_logit[t].
  - Per-core device output: per-horizon partial sums [sum_t log S, sum_t pos];
    host combines the 8 cores (the scalar "all-reduce") and applies the
    1/(Nk) and 1/n_pred factors.

Device pipeline per horizon k (Tk = 255-kk rows on this core):
  zhT[e,t]   = sum_c WT[k][c,e] * ctxT[c,t]          (PE, psum f32 -> bf16 sbuf)
  sumsq/pos  = ones-matmul partition reductions of zh^2 and zh*encbT_n
               (muls on GpSimd, reduction on PE)
  q          = exp(-0.5*Ln(ss) + ln(1/temp))         (ACT only — keeps the
               Exp/Ln/Copy table set loaded; no Sqrt set switch)
  L          = zhT.T @ poolT_n                       (PE, 4 x 512-col chunks)
  E          = exp(q * L)                            (ACT, psum -> bf16 sbuf)
  S          = sum_m E * cnt                         (DVE affine_mul_reduce)
  logS       = Ln(S); sum_t via ones-matmul          (ACT + PE)
"""

import math

import numpy as np
import ml_dtypes

import concourse.bass as bass
import concourse.tile as tile
import concourse.mybir as mybir
from concourse import bacc
from concourse import bass_utils

B, T, C_CTX, C_ENC, NPRED, K_NEG = 8, 256, 512, 256, 12, 128
POOL = B * T  # 2048
TEMP = 0.07
N_CORES = 8

DT = mybir.dt
BF16 = ml_dtypes.bfloat16
FP8 = ml_dtypes.float8_e4m3

_CACHE = {}


class _OneActSetBacc(bacc.Bacc):
    """Bacc whose act-table pass sees only the natural_log_exp_and_others
    set (it contains Exp, Ln, Copy, Identity, Square — everything this
    kernel uses), so exactly one ACT_TABLE_LOAD is emitted instead of a
    reload on every Exp<->Ln<->Copy transition (1.3us each). Set indices
    are preserved; the other sets are just emptied so they are never
    chosen."""

    def insert_act_table_loads(self):
        import bass_rust as _bass_rust
        from concourse.hw_specs import get_activation_tables

        has_activation = any(
            isinstance(i, mybir.InstActivation)
            for b in self.main_func.blocks
            for i in b.instructions
        )
        if not has_activation:
            return
        keep = "natural_log_exp_and_others"
        tables = [
            (name, funcs if name == keep else set())
            for name, funcs in get_activation_tables(self.m.arch).items()
        ]
        _bass_rust.insert_act_table_loads(self, tables)


def _build_program():
    nc = _OneActSetBacc(
        "TRN2",
        target_bir_lowering=False,
        debug=False,
        enable_asserts=False,
        num_devices=N_CORES,
    )
    ctxT = nc.dram_tensor("ctxT", [C_CTX, T], DT.bfloat16, kind="ExternalInput").ap()
    WT = nc.dram_tensor("WT", [NPRED, C_CTX, C_ENC], DT.bfloat16, kind="ExternalInput").ap()
    poolT = nc.dram_tensor("poolT", [C_ENC, POOL], DT.bfloat16, kind="ExternalInput").ap()
    encbT = nc.dram_tensor("encbT", [C_ENC, T], DT.bfloat16, kind="ExternalInput").ap()
    cnt = nc.dram_tensor("cnt", [NPRED, T, POOL], DT.float8e4, kind="ExternalInput").ap()
    out_sums = nc.dram_tensor("out_sums", [1, 2 * NPRED], DT.float32, kind="ExternalOutput").ap()

    with tile.TileContext(nc) as tc:
        _emit(nc, tc, ctxT, WT, poolT, encbT, cnt, out_sums)
    nc.compile()
    return nc


def _emit(nc, tc, ctxT, WT, poolT, encbT, cnt, out_sums):
    import contextlib

    f32 = DT.float32
    bf16 = DT.bfloat16
    AF = mybir.ActivationFunctionType
    ALU = mybir.AluOpType

    ctx = contextlib.ExitStack()
    with ctx:
        singles = ctx.enter_context(tc.tile_pool(name="singles", bufs=1))
        wt_pool = ctx.enter_context(tc.tile_pool(name="wt", bufs=3))
        zh_pool = ctx.enter_context(tc.tile_pool(name="zh", bufs=2))
        big_pool = ctx.enter_context(tc.tile_pool(name="big", bufs=3))
        e_pool = ctx.enter_context(tc.tile_pool(name="epool", bufs=3))
        cnt_pool = ctx.enter_context(tc.tile_pool(name="cntp", bufs=4))
        tout_pool = ctx.enter_context(tc.tile_pool(name="tout", bufs=3))
        sv_pool = ctx.enter_context(tc.tile_pool(name="sv", bufs=8))
        qcol_pool = ctx.enter_context(tc.tile_pool(name="qcol", bufs=2))
        rows_pool = ctx.enter_context(tc.tile_pool(name="rows", bufs=3))
        psum_L = ctx.enter_context(tc.tile_pool(name="psL", bufs=3, space="PSUM"))
        psum_zh = ctx.enter_context(tc.tile_pool(name="psz", bufs=1, space="PSUM"))
        psum_sp = ctx.enter_context(tc.tile_pool(name="pssp", bufs=1, space="PSUM"))
        dram = ctx.enter_context(tc.tile_pool(name="dram", bufs=1, space="DRAM"))

        # ---- one-time loads ----
        ctx_sb = []
        for cc in range(4):
            t_ = singles.tile([128, T], bf16, tag=f"ctx{cc}")
            nc.sync.dma_start(out=t_, in_=ctxT[cc * 128:(cc + 1) * 128, :])
            ctx_sb.append(t_)
        pool_sb = []
        encb_sb = []
        for eh in range(2):
            p_ = singles.tile([128, POOL], bf16, tag=f"pool{eh}")
            nc.sync.dma_start(out=p_, in_=poolT[eh * 128:(eh + 1) * 128, :])
            pool_sb.append(p_)
            e_ = singles.tile([128, T], bf16, tag=f"encb{eh}")
            nc.sync.dma_start(out=e_, in_=encbT[eh * 128:(eh + 1) * 128, :])
            encb_sb.append(e_)

        ones_bf = singles.tile([128, 1], bf16, tag="ones_bf")
        nc.vector.memset(ones_bf, 1.0)
        ones_f32 = singles.tile([128, 1], f32, tag="ones_f32")
        nc.vector.memset(ones_f32, 1.0)
        outbuf = singles.tile([1, 2 * NPRED], f32, tag="outbuf")
        coll = singles.tile([128, 4 * NPRED], f32, tag="coll")
        nc.vector.memset(coll, 0.0)
        warm_rhs = singles.tile([128, 512], bf16, tag="warm_rhs")
        nc.vector.memset(warm_rhs, 0.0)
        biasq = singles.tile([128, 1], f32, tag="biasq")
        nc.vector.memset(biasq, float(math.log(1.0 / TEMP)))


        # ---- phase 1: all horizons' zh, norms, pos sums, and q columns ----
        zh_all = {}
        qcol_all = {}
        # warm the PE clock gate (HAM) with throwaway matmuls while the
        # input DMAs are in flight; ~4us of PE activity flips K to 8/8.
        wps = psum_zh.tile([128, 2, 255], f32, tag="zhps", name="warm_ps")
        for i in range(10):
            nc.tensor.matmul(wps[:1, 0, :255], ones_bf, warm_rhs[:, :255],
                             start=(i == 0), stop=(i == 9))

        def emit_p1(kk):
            Tk = T - 1 - kk  # 255 - kk rows on this core
            R = [128, Tk - 128]

            # W[k].T in one DMA: [c(4x128), e 256] -> [128, 4, 256]
            wtile = wt_pool.tile([128, 4, C_ENC], bf16, tag="wt")
            nc.sync.dma_start(
                out=wtile,
                in_=WT[kk].rearrange("(cc p) e -> p cc e", p=128),
            )

            # matmul1: zhT[e, t] (2 e-halves in one psum bank)
            zps = psum_zh.tile([128, 2, Tk], f32, tag="zhps")
            for eh in range(2):
                for cc in range(4):
                    nc.tensor.matmul(
                        zps[:, eh, :],
                        wtile[:, cc, eh * 128:(eh + 1) * 128],
                        ctx_sb[cc][:, :Tk],
                        start=(cc == 0),
                        stop=(cc == 3),
                    )
            z_ = zh_pool.tile([128, 2, Tk], bf16, tag=f"zh{kk}", name=f"zh{kk}")
            nc.scalar.activation(out=z_, in_=zps, func=AF.Copy)
            zh_sb = [z_[:, 0, :], z_[:, 1, :]]
            zh_all[kk] = zh_sb

            # zh^2 and zh*pos packed into one [128, 2*Tk] tile per e-half
            bgs = []
            for eh in range(2):
                bg = big_pool.tile([128, 2 * Tk], bf16, tag=f"big{eh}")
                nc.vector.tensor_mul(bg[:, :Tk], zh_sb[eh], zh_sb[eh])
                nc.vector.tensor_mul(
                    bg[:, Tk:2 * Tk], zh_sb[eh], encb_sb[eh][:, kk + 1:kk + 1 + Tk]
                )
                bgs.append(bg)

            # per t-half: ss and pos column vectors via tiny ones-rhs matmuls
            # (bg as lhsT: out[t, 1] = sum_e bg[e, t]); then
            # q = 1/(sqrt(ss)*TEMP) = exp(-0.5*ln(ss) + ln(1/TEMP)) on columns.
            q_col = []
            for h in range(2):
                Rh = R[h]
                sp2 = psum_sp.tile([128, 2], f32, tag="sp", name=f"sp{kk}_{h}")
                for eh in range(2):
                    nc.tensor.matmul(
                        sp2[:Rh, 0:1],
                        bgs[eh][:, h * 128:h * 128 + Rh],
                        ones_bf,
                        start=(eh == 0),
                        stop=(eh == 1),
                    )
                for eh in range(2):
                    nc.tensor.matmul(
                        sp2[:Rh, 1:2],
                        bgs[eh][:, Tk + h * 128:Tk + h * 128 + Rh],
                        ones_bf,
                        start=(eh == 0),
                        stop=(eh == 1),
                    )
                lnc = sv_pool.tile([128, 1], f32, tag="lnc")
                nc.scalar.activation(out=lnc[:Rh, :], in_=sp2[:Rh, 0:1], func=AF.Ln)
                qc = qcol_pool.tile([128, 1], f32, tag=f"qc{kk}_{h}", name=f"qc{kk}_{h}")
                nc.scalar.activation(
                    out=qc[:Rh, :], in_=lnc[:Rh, :], func=AF.Exp,
                    scale=-0.5, bias=biasq[:Rh, :],
                )
                # pos~ column straight into the collection tile (cols 24..47)
                nc.vector.tensor_mul(
                    coll[:Rh, 2 * NPRED + 2 * kk + h:2 * NPRED + 2 * kk + h + 1],
                    sp2[:Rh, 1:2],
                    qc[:Rh, :],
                )
                q_col.append(qc[:Rh, :])
            qcol_all[kk] = q_col

        # ---- phase 2: full-pool logits, exp, masked reduce per horizon ----
        def emit_p2(kk):
            Tk = T - 1 - kk
            R = [128, Tk - 128]
            zh_sb = zh_all[kk]
            q_col = qcol_all[kk]

            for h in range(2):
                Rh = R[h]
                cb = cnt_pool.tile([128, POOL], DT.float8e4, tag="cnt")
                nc.sync.dma_start(
                    out=cb[:Rh, :],
                    in_=cnt[kk, h * 128:h * 128 + Rh, :],
                )
                ee = e_pool.tile([128, POOL], bf16, tag="E")
                for g in range(2):
                    lp = psum_L.tile([128, 2, 512], f32, tag="L", name=f"lp{g}")
                    for i in range(2):
                        mc = 2 * g + i
                        for eh in range(2):
                            nc.tensor.matmul(
                                lp[:Rh, i, :],
                                zh_sb[eh][:, h * 128:h * 128 + Rh],
                                pool_sb[eh][:, mc * 512:(mc + 1) * 512],
                                start=(eh == 0),
                                stop=(eh == 1),
                            )
                    nc.scalar.activation(
                        out=ee[:Rh, g * 1024:(g + 1) * 1024],
                        in_=lp[:Rh, :, :],
                        func=AF.Exp,
                        scale=q_col[h],
                    )
                to_ = tout_pool.tile([128, POOL], DT.float8e4, tag="to")
                s_cur = sv_pool.tile([128, 1], f32, tag="scur")
                nc.vector.affine_mul_reduce(
                    out=to_[:Rh, :],
                    accum_out=s_cur[:Rh, :],
                    in0=ee[:Rh, :],
                    in1=cb[:Rh, :],
                    scale=1.0,
                    bias=0.0,
                )
                j = 2 * kk + h
                nc.scalar.activation(
                    out=coll[:Rh, j:j + 1], in_=s_cur[:Rh, :], func=AF.Ln
                )

        for kk in range(NPRED):
            emit_p1(kk)
        for kk in range(NPRED):
            emit_p2(kk)

        fin = psum_sp.tile([1, 4 * NPRED], f32, tag="sp", name="fin")
        nc.tensor.matmul(fin, ones_f32, coll, start=True, stop=True)
        fin_sb = singles.tile([1, 4 * NPRED], f32, tag="fin_sb")
        nc.vector.tensor_copy(out=fin_sb, in_=fin)
        nc.vector.tensor_add(
            outbuf[:, 0:2 * NPRED],
            fin_sb[:, 0:4 * NPRED:2],
            fin_sb[:, 1:4 * NPRED:2],
        )
        nc.sync.dma_start(out=out_sums, in_=outbuf)


def _build_cnt(neg_idx):
    """cnt[b, kk, t, m] (uint8): negative multiplicities + 1 at the positive."""
    cnt = np.zeros((B, NPRED, T, POOL), dtype=np.uint8)
    b_ar = np.arange(B, dtype=np.int64)[:, None]
    for kk in range(NPRED):
        Tk = T - 1 - kk
        Nk = B * Tk
        idx = np.asarray(neg_idx[kk, :Nk], dtype=np.int64)  # [Nk, K]
        flat = idx + np.arange(Nk, dtype=np.int64)[:, None] * POOL
        c = np.bincount(flat.ravel(), minlength=Nk * POOL)
        c = c.astype(np.uint8).reshape(B, Tk, POOL)
        t_ar = np.arange(Tk, dtype=np.int64)[None, :]
        pos_m = T * b_ar + (kk + 1) + t_ar
        c[b_ar, t_ar, pos_m] += 1
        cnt[:, kk, :Tk, :] = c
    return cnt


def kernel(context, encoded, W, neg_idx):
    context = np.asarray(context, dtype=np.float32)
    encoded = np.asarray(encoded, dtype=np.float32)
    W = np.asarray(W, dtype=np.float32)
    neg_idx = np.asarray(neg_idx)

    if "nc" not in _CACHE:
        _CACHE["nc"] = _build_program()
    nc = _CACHE["nc"]

    # host prep: layout transposes, normalized pool, count mask
    pool = encoded.reshape(POOL, C_ENC)
    pool_n = pool / np.linalg.norm(pool, axis=-1, keepdims=True)
    poolT_bf = np.ascontiguousarray(pool_n.T).astype(BF16)  # [256, 2048]
    WT_bf = np.ascontiguousarray(W.transpose(0, 2, 1)).astype(BF16)  # [12, 512, 256]
    cnt_u8 = _build_cnt(neg_idx)
    fp8_lut = np.arange(256).astype(np.float32).astype(FP8)

    in_maps = []
    for b in range(N_CORES):
        in_maps.append(
            {
                "ctxT": np.ascontiguousarray(context[b].T).astype(BF16),
                "WT": WT_bf,
                "poolT": poolT_bf,
                "encbT": np.ascontiguousarray(poolT_bf[:, T * b:T * (b + 1)]),
                "cnt": fp8_lut[cnt_u8[b]],
            }
        )

    _CACHE["in_maps"] = in_maps
    res = bass_utils.run_bass_kernel_spmd(nc, in_maps, core_ids=list(range(N_CORES)))

    total = np.float64(0.0)
    for kk in range(NPRED):
        Tk = T - 1 - kk
        num = np.float64(0.0)
        for b in range(N_CORES):
            sums = res.results[b]["out_sums"][0]
            num += np.float64(sums[kk]) - np.float64(sums[NPRED + kk])
        total += num / (B * Tk)
    total = total / NPRED
    return np.float32(total)


# revision 36
# speedup vs baseline: 1.1296x; 1.0588x over previous
"""CPC criterion (contrastive predictive coding loss) on 8 TRN2 NeuronCores.

Strategy (data-parallel over batch, per sharding hint):
  - Core b handles batch element b (B == 8 == n_cores).
  - The per-row gather of K=128 negatives has no efficient per-partition
    gather primitive on TRN2, so it is reformulated as a dense masked
    reduction:  S[t] = sum_m cnt[t, m] * exp(q_t * L[t, m])
    where L = zh_unnorm @ pool_norm.T is the full-pool logit matrix
    (a PE-friendly matmul), q_t = 1/(|zh_t| * temperature), and
    cnt[t, m] (built on the host from neg_idx) counts how many times pool
    entry m appears among row t's negatives, +1 at the positive target.
    Then lse[t] = log S[t], and loss rows are lse[t] - pos_logit[t].
  - Per-core device output: per-horizon partial sums [sum_t log S, sum_t pos];
    host combines the 8 cores (the scalar "all-reduce") and applies the
    1/(Nk) and 1/n_pred factors.

Device pipeline per horizon k (Tk = 255-kk rows on this core):
  zhT[e,t]   = sum_c WT[k][c,e] * ctxT[c,t]          (PE, psum f32 -> bf16 sbuf)
  sumsq/pos  = ones-matmul partition reductions of zh^2 and zh*encbT_n
               (muls on GpSimd, reduction on PE)
  q          = exp(-0.5*Ln(ss) + ln(1/temp))         (ACT only — keeps the
               Exp/Ln/Copy table set loaded; no Sqrt set switch)
  L          = zhT.T @ poolT_n                       (PE, 4 x 512-col chunks)
  E          = exp(q * L)                            (ACT, psum -> bf16 sbuf)
  S          = sum_m E * cnt                         (DVE affine_mul_reduce)
  logS       = Ln(S); sum_t via ones-matmul          (ACT + PE)
"""

import math

import numpy as np
import ml_dtypes

import concourse.bass as bass
import concourse.tile as tile
import concourse.mybir as mybir
from concourse import bacc
from concourse import bass_utils

B, T, C_CTX, C_ENC, NPRED, K_NEG = 8, 256, 512, 256, 12, 128
POOL = B * T  # 2048
TEMP = 0.07
N_CORES = 8

DT = mybir.dt
BF16 = ml_dtypes.bfloat16
FP8 = ml_dtypes.float8_e4m3

_CACHE = {}


class _OneActSetBacc(bacc.Bacc):
    """Bacc whose act-table pass sees only the natural_log_exp_and_others
    set (it contains Exp, Ln, Copy, Identity, Square — everything this
    kernel uses), so exactly one ACT_TABLE_LOAD is emitted instead of a
    reload on every Exp<->Ln<->Copy transition (1.3us each). Set indices
    are preserved; the other sets are just emptied so they are never
    chosen."""

    def insert_act_table_loads(self):
        import bass_rust as _bass_rust
        from concourse.hw_specs import get_activation_tables

        has_activation = any(
            isinstance(i, mybir.InstActivation)
            for b in self.main_func.blocks
            for i in b.instructions
        )
        if not has_activation:
            return
        keep = "natural_log_exp_and_others"
        tables = [
            (name, funcs if name == keep else set())
            for name, funcs in get_activation_tables(self.m.arch).items()
        ]
        _bass_rust.insert_act_table_loads(self, tables)


def _build_program():
    nc = _OneActSetBacc(
        "TRN2",
        target_bir_lowering=False,
        debug=False,
        enable_asserts=False,
        num_devices=N_CORES,
    )
    ctxT = nc.dram_tensor("ctxT", [C_CTX, T], DT.bfloat16, kind="ExternalInput").ap()
    WT = nc.dram_tensor("WT", [NPRED, C_CTX, C_ENC], DT.bfloat16, kind="ExternalInput").ap()
    poolT = nc.dram_tensor("poolT", [C_ENC, POOL], DT.bfloat16, kind="ExternalInput").ap()
    encbT = nc.dram_tensor("encbT", [C_ENC, T], DT.bfloat16, kind="ExternalInput").ap()
    cnt = nc.dram_tensor("cnt", [NPRED, T, POOL], DT.float8e4, kind="ExternalInput").ap()
    out_sums = nc.dram_tensor("out_sums", [1, 2 * NPRED], DT.float32, kind="ExternalOutput").ap()

    with tile.TileContext(nc) as tc:
        _emit(nc, tc, ctxT, WT, poolT, encbT, cnt, out_sums)
    nc.compile()
    return nc


def _emit(nc, tc, ctxT, WT, poolT, encbT, cnt, out_sums):
    import contextlib

    f32 = DT.float32
    bf16 = DT.bfloat16
    AF = mybir.ActivationFunctionType
    ALU = mybir.AluOpType

    ctx = contextlib.ExitStack()
    with ctx:
        singles = ctx.enter_context(tc.tile_pool(name="singles", bufs=1))
        wt_pool = ctx.enter_context(tc.tile_pool(name="wt", bufs=3))
        zh_pool = ctx.enter_context(tc.tile_pool(name="zh", bufs=2))
        big_pool = ctx.enter_context(tc.tile_pool(name="big", bufs=3))
        e_pool = ctx.enter_context(tc.tile_pool(name="epool", bufs=3))
        cnt_pool = ctx.enter_context(tc.tile_pool(name="cntp", bufs=4))
        tout_pool = ctx.enter_context(tc.tile_pool(name="tout", bufs=3))
        sv_pool = ctx.enter_context(tc.tile_pool(name="sv", bufs=8))
        qcol_pool = ctx.enter_context(tc.tile_pool(name="qcol", bufs=2))
        rows_pool = ctx.enter_context(tc.tile_pool(name="rows", bufs=3))
        psum_L = ctx.enter_context(tc.tile_pool(name="psL", bufs=3, space="PSUM"))
        psum_zh = ctx.enter_context(tc.tile_pool(name="psz", bufs=1, space="PSUM"))
        psum_sp = ctx.enter_context(tc.tile_pool(name="pssp", bufs=1, space="PSUM"))
        dram = ctx.enter_context(tc.tile_pool(name="dram", bufs=1, space="DRAM"))

        # ---- one-time loads ----
        ctx_sb = []
        for cc in range(4):
            t_ = singles.tile([128, T], bf16, tag=f"ctx{cc}")
            nc.sync.dma_start(out=t_, in_=ctxT[cc * 128:(cc + 1) * 128, :])
            ctx_sb.append(t_)
        pool_sb = []
        encb_sb = []
        for eh in range(2):
            p_ = singles.tile([128, POOL], bf16, tag=f"pool{eh}")
            nc.sync.dma_start(out=p_, in_=poolT[eh * 128:(eh + 1) * 128, :])
            pool_sb.append(p_)
            e_ = singles.tile([128, T], bf16, tag=f"encb{eh}")
            nc.sync.dma_start(out=e_, in_=encbT[eh * 128:(eh + 1) * 128, :])
            encb_sb.append(e_)

        ones_bf = singles.tile([128, 1], bf16, tag="ones_bf")
        nc.vector.memset(ones_bf, 1.0)
        ones_f32 = singles.tile([128, 1], f32, tag="ones_f32")
        nc.vector.memset(ones_f32, 1.0)
        outbuf = singles.tile([1, 2 * NPRED], f32, tag="outbuf")
        coll = singles.tile([128, 4 * NPRED], f32, tag="coll")
        nc.vector.memset(coll, 0.0)
        warm_rhs = singles.tile([128, 512], bf16, tag="warm_rhs")
        nc.vector.memset(warm_rhs, 0.0)
        biasq = singles.tile([128, 1], f32, tag="biasq")
        nc.vector.memset(biasq, float(math.log(1.0 / TEMP)))


        # ---- phase 1: all horizons' zh, norms, pos sums, and q columns ----
        zh_all = {}
        qcol_all = {}
        # warm the PE clock gate (HAM) with throwaway matmuls while the
        # input DMAs are in flight; ~4us of PE activity flips K to 8/8.
        wps = psum_zh.tile([128, 2, 255], f32, tag="zhps", name="warm_ps")
        for i in range(10):
            nc.tensor.matmul(wps[:1, 0, :255], ones_bf, warm_rhs[:, :255],
                             start=(i == 0), stop=(i == 9))

        def emit_p1(kk):
            Tk = T - 1 - kk  # 255 - kk rows on this core
            R = [128, Tk - 128]

            # W[k].T in one DMA: [c(4x128), e 256] -> [128, 4, 256]
            wtile = wt_pool.tile([128, 4, C_ENC], bf16, tag="wt")
            nc.gpsimd.dma_start(
                out=wtile,
                in_=WT[kk].rearrange("(cc p) e -> p cc e", p=128),
            )

            # matmul1: zhT[e, t] (2 e-halves in one psum bank)
            zps = psum_zh.tile([128, 2, Tk], f32, tag="zhps")
            for eh in range(2):
                for cc in range(4):
                    nc.tensor.matmul(
                        zps[:, eh, :],
                        wtile[:, cc, eh * 128:(eh + 1) * 128],
                        ctx_sb[cc][:, :Tk],
                        start=(cc == 0),
                        stop=(cc == 3),
                    )
            z_ = zh_pool.tile([128, 2, Tk], bf16, tag=f"zh{kk}", name=f"zh{kk}")
            half = Tk // 2
            nc.scalar.activation(
                out=z_[:, :, :half], in_=zps[:, :, :half], func=AF.Copy)
            nc.vector.tensor_copy(out=z_[:, :, half:], in_=zps[:, :, half:])
            zh_sb = [z_[:, 0, :], z_[:, 1, :]]
            zh_all[kk] = zh_sb

            # zh^2 and zh*pos packed into one [128, 2*Tk] tile per e-half
            bgs = []
            for eh in range(2):
                bg = big_pool.tile([128, 2 * Tk], bf16, tag=f"big{eh}")
                nc.vector.tensor_mul(bg[:, :Tk], zh_sb[eh], zh_sb[eh])
                nc.vector.tensor_mul(
                    bg[:, Tk:2 * Tk], zh_sb[eh], encb_sb[eh][:, kk + 1:kk + 1 + Tk]
                )
                bgs.append(bg)

            # per t-half: ss and pos column vectors via tiny ones-rhs matmuls
            # (bg as lhsT: out[t, 1] = sum_e bg[e, t]); then
            # q = 1/(sqrt(ss)*TEMP) = exp(-0.5*ln(ss) + ln(1/TEMP)) on columns.
            q_col = []
            for h in range(2):
                Rh = R[h]
                sp2 = psum_sp.tile([128, 2], f32, tag="sp", name=f"sp{kk}_{h}")
                for eh in range(2):
                    nc.tensor.matmul(
                        sp2[:Rh, 0:1],
                        bgs[eh][:, h * 128:h * 128 + Rh],
                        ones_bf,
                        start=(eh == 0),
                        stop=(eh == 1),
                    )
                for eh in range(2):
                    nc.tensor.matmul(
                        sp2[:Rh, 1:2],
                        bgs[eh][:, Tk + h * 128:Tk + h * 128 + Rh],
                        ones_bf,
                        start=(eh == 0),
                        stop=(eh == 1),
                    )
                lnc = sv_pool.tile([128, 1], f32, tag="lnc")
                nc.scalar.activation(out=lnc[:Rh, :], in_=sp2[:Rh, 0:1], func=AF.Ln)
                qc = qcol_pool.tile([128, 1], f32, tag=f"qc{kk}_{h}", name=f"qc{kk}_{h}")
                nc.scalar.activation(
                    out=qc[:Rh, :], in_=lnc[:Rh, :], func=AF.Exp,
                    scale=-0.5, bias=biasq[:Rh, :],
                )
                # pos~ column straight into the collection tile (cols 24..47)
                nc.vector.tensor_mul(
                    coll[:Rh, 2 * NPRED + 2 * kk + h:2 * NPRED + 2 * kk + h + 1],
                    sp2[:Rh, 1:2],
                    qc[:Rh, :],
                )
                q_col.append(qc[:Rh, :])
            qcol_all[kk] = q_col

        # ---- phase 2: full-pool logits, exp, masked reduce per horizon ----
        def emit_p2(kk):
            Tk = T - 1 - kk
            R = [128, Tk - 128]
            zh_sb = zh_all[kk]
            q_col = qcol_all[kk]

            for h in range(2):
                Rh = R[h]
                cb = cnt_pool.tile([128, POOL], DT.float8e4, tag="cnt")
                nc.sync.dma_start(
                    out=cb[:Rh, :],
                    in_=cnt[kk, h * 128:h * 128 + Rh, :],
                )
                ee = e_pool.tile([128, POOL], bf16, tag="E")
                for g in range(2):
                    lp = psum_L.tile([128, 2, 512], f32, tag="L", name=f"lp{g}")
                    for i in range(2):
                        mc = 2 * g + i
                        for eh in range(2):
                            nc.tensor.matmul(
                                lp[:Rh, i, :],
                                zh_sb[eh][:, h * 128:h * 128 + Rh],
                                pool_sb[eh][:, mc * 512:(mc + 1) * 512],
                                start=(eh == 0),
                                stop=(eh == 1),
                            )
                    nc.scalar.activation(
                        out=ee[:Rh, g * 1024:(g + 1) * 1024],
                        in_=lp[:Rh, :, :],
                        func=AF.Exp,
                        scale=q_col[h],
                    )
                to_ = tout_pool.tile([128, POOL], DT.float8e4, tag="to")
                s_cur = sv_pool.tile([128, 1], f32, tag="scur")
                nc.vector.affine_mul_reduce(
                    out=to_[:Rh, :],
                    accum_out=s_cur[:Rh, :],
                    in0=ee[:Rh, :],
                    in1=cb[:Rh, :],
                    scale=1.0,
                    bias=0.0,
                )
                j = 2 * kk + h
                nc.scalar.activation(
                    out=coll[:Rh, j:j + 1], in_=s_cur[:Rh, :], func=AF.Ln
                )

        for kk in range(NPRED):
            emit_p1(kk)
        for kk in range(NPRED):
            emit_p2(kk)

        fin = psum_sp.tile([1, 4 * NPRED], f32, tag="sp", name="fin")
        nc.tensor.matmul(fin, ones_f32, coll, start=True, stop=True)
        fin_sb = singles.tile([1, 4 * NPRED], f32, tag="fin_sb")
        nc.vector.tensor_copy(out=fin_sb, in_=fin)
        nc.vector.tensor_add(
            outbuf[:, 0:2 * NPRED],
            fin_sb[:, 0:4 * NPRED:2],
            fin_sb[:, 1:4 * NPRED:2],
        )
        nc.sync.dma_start(out=out_sums, in_=outbuf)


def _build_cnt(neg_idx):
    """cnt[b, kk, t, m] (uint8): negative multiplicities + 1 at the positive."""
    cnt = np.zeros((B, NPRED, T, POOL), dtype=np.uint8)
    b_ar = np.arange(B, dtype=np.int64)[:, None]
    for kk in range(NPRED):
        Tk = T - 1 - kk
        Nk = B * Tk
        idx = np.asarray(neg_idx[kk, :Nk], dtype=np.int64)  # [Nk, K]
        flat = idx + np.arange(Nk, dtype=np.int64)[:, None] * POOL
        c = np.bincount(flat.ravel(), minlength=Nk * POOL)
        c = c.astype(np.uint8).reshape(B, Tk, POOL)
        t_ar = np.arange(Tk, dtype=np.int64)[None, :]
        pos_m = T * b_ar + (kk + 1) + t_ar
        c[b_ar, t_ar, pos_m] += 1
        cnt[:, kk, :Tk, :] = c
    return cnt


def kernel(context, encoded, W, neg_idx):
    context = np.asarray(context, dtype=np.float32)
    encoded = np.asarray(encoded, dtype=np.float32)
    W = np.asarray(W, dtype=np.float32)
    neg_idx = np.asarray(neg_idx)

    if "nc" not in _CACHE:
        _CACHE["nc"] = _build_program()
    nc = _CACHE["nc"]

    # host prep: layout transposes, normalized pool, count mask
    pool = encoded.reshape(POOL, C_ENC)
    pool_n = pool / np.linalg.norm(pool, axis=-1, keepdims=True)
    poolT_bf = np.ascontiguousarray(pool_n.T).astype(BF16)  # [256, 2048]
    WT_bf = np.ascontiguousarray(W.transpose(0, 2, 1)).astype(BF16)  # [12, 512, 256]
    cnt_u8 = _build_cnt(neg_idx)
    fp8_lut = np.arange(256).astype(np.float32).astype(FP8)

    in_maps = []
    for b in range(N_CORES):
        in_maps.append(
            {
                "ctxT": np.ascontiguousarray(context[b].T).astype(BF16),
                "WT": WT_bf,
                "poolT": poolT_bf,
                "encbT": np.ascontiguousarray(poolT_bf[:, T * b:T * (b + 1)]),
                "cnt": fp8_lut[cnt_u8[b]],
            }
        )

    _CACHE["in_maps"] = in_maps
    res = bass_utils.run_bass_kernel_spmd(nc, in_maps, core_ids=list(range(N_CORES)))

    total = np.float64(0.0)
    for kk in range(NPRED):
        Tk = T - 1 - kk
        num = np.float64(0.0)
        for b in range(N_CORES):
            sums = res.results[b]["out_sums"][0]
            num += np.float64(sums[kk]) - np.float64(sums[NPRED + kk])
        total += num / (B * Tk)
    total = total / NPRED
    return np.float32(total)


# revision 37
# speedup vs baseline: 1.1578x; 1.0250x over previous
"""CPC criterion (contrastive predictive coding loss) on 8 TRN2 NeuronCores.

Strategy (data-parallel over batch, per sharding hint):
  - Core b handles batch element b (B == 8 == n_cores).
  - The per-row gather of K=128 negatives has no efficient per-partition
    gather primitive on TRN2, so it is reformulated as a dense masked
    reduction:  S[t] = sum_m cnt[t, m] * exp(q_t * L[t, m])
    where L = zh_unnorm @ pool_norm.T is the full-pool logit matrix
    (a PE-friendly matmul), q_t = 1/(|zh_t| * temperature), and
    cnt[t, m] (built on the host from neg_idx) counts how many times pool
    entry m appears among row t's negatives, +1 at the positive target.
    Then lse[t] = log S[t], and loss rows are lse[t] - pos_logit[t].
  - Per-core device output: per-horizon partial sums [sum_t log S, sum_t pos];
    host combines the 8 cores (the scalar "all-reduce") and applies the
    1/(Nk) and 1/n_pred factors.

Device pipeline per horizon k (Tk = 255-kk rows on this core):
  zhT[e,t]   = sum_c WT[k][c,e] * ctxT[c,t]          (PE, psum f32 -> bf16 sbuf)
  sumsq/pos  = ones-matmul partition reductions of zh^2 and zh*encbT_n
               (muls on GpSimd, reduction on PE)
  q          = exp(-0.5*Ln(ss) + ln(1/temp))         (ACT only — keeps the
               Exp/Ln/Copy table set loaded; no Sqrt set switch)
  L          = zhT.T @ poolT_n                       (PE, 4 x 512-col chunks)
  E          = exp(q * L)                            (ACT, psum -> bf16 sbuf)
  S          = sum_m E * cnt                         (DVE affine_mul_reduce)
  logS       = Ln(S); sum_t via ones-matmul          (ACT + PE)
"""

import math

import numpy as np
import ml_dtypes

import concourse.bass as bass
import concourse.tile as tile
import concourse.mybir as mybir
from concourse import bacc
from concourse import bass_utils

B, T, C_CTX, C_ENC, NPRED, K_NEG = 8, 256, 512, 256, 12, 128
POOL = B * T  # 2048
TEMP = 0.07
N_CORES = 8

DT = mybir.dt
BF16 = ml_dtypes.bfloat16
FP8 = ml_dtypes.float8_e4m3

_CACHE = {}


class _OneActSetBacc(bacc.Bacc):
    """Bacc whose act-table pass sees only the natural_log_exp_and_others
    set (it contains Exp, Ln, Copy, Identity, Square — everything this
    kernel uses), so exactly one ACT_TABLE_LOAD is emitted instead of a
    reload on every Exp<->Ln<->Copy transition (1.3us each). Set indices
    are preserved; the other sets are just emptied so they are never
    chosen."""

    def insert_act_table_loads(self):
        import bass_rust as _bass_rust
        from concourse.hw_specs import get_activation_tables

        has_activation = any(
            isinstance(i, mybir.InstActivation)
            for b in self.main_func.blocks
            for i in b.instructions
        )
        if not has_activation:
            return
        keep = "natural_log_exp_and_others"
        tables = [
            (name, funcs if name == keep else set())
            for name, funcs in get_activation_tables(self.m.arch).items()
        ]
        _bass_rust.insert_act_table_loads(self, tables)


def _build_program():
    nc = _OneActSetBacc(
        "TRN2",
        target_bir_lowering=False,
        debug=False,
        enable_asserts=False,
        num_devices=N_CORES,
    )
    ctxT = nc.dram_tensor("ctxT", [C_CTX, T], DT.bfloat16, kind="ExternalInput").ap()
    WT = nc.dram_tensor("WT", [NPRED, C_CTX, C_ENC], DT.bfloat16, kind="ExternalInput").ap()
    poolT = nc.dram_tensor("poolT", [C_ENC, POOL], DT.bfloat16, kind="ExternalInput").ap()
    encbT = nc.dram_tensor("encbT", [C_ENC, T], DT.bfloat16, kind="ExternalInput").ap()
    cnt = nc.dram_tensor("cnt", [NPRED, T, POOL], DT.float8e4, kind="ExternalInput").ap()
    out_sums = nc.dram_tensor("out_sums", [1, 2 * NPRED], DT.float32, kind="ExternalOutput").ap()

    with tile.TileContext(nc) as tc:
        _emit(nc, tc, ctxT, WT, poolT, encbT, cnt, out_sums)
    nc.compile()
    return nc


def _emit(nc, tc, ctxT, WT, poolT, encbT, cnt, out_sums):
    import contextlib

    f32 = DT.float32
    bf16 = DT.bfloat16
    AF = mybir.ActivationFunctionType
    ALU = mybir.AluOpType

    ctx = contextlib.ExitStack()
    with ctx:
        singles = ctx.enter_context(tc.tile_pool(name="singles", bufs=1))
        wt_pool = ctx.enter_context(tc.tile_pool(name="wt", bufs=3))
        zh_pool = ctx.enter_context(tc.tile_pool(name="zh", bufs=2))
        big_pool = ctx.enter_context(tc.tile_pool(name="big", bufs=3))
        e_pool = ctx.enter_context(tc.tile_pool(name="epool", bufs=3))
        cnt_pool = ctx.enter_context(tc.tile_pool(name="cntp", bufs=4))
        tout_pool = ctx.enter_context(tc.tile_pool(name="tout", bufs=3))
        sv_pool = ctx.enter_context(tc.tile_pool(name="sv", bufs=8))
        qcol_pool = ctx.enter_context(tc.tile_pool(name="qcol", bufs=2))
        rows_pool = ctx.enter_context(tc.tile_pool(name="rows", bufs=3))
        psum_L = ctx.enter_context(tc.tile_pool(name="psL", bufs=3, space="PSUM"))
        psum_zh = ctx.enter_context(tc.tile_pool(name="psz", bufs=1, space="PSUM"))
        psum_sp = ctx.enter_context(tc.tile_pool(name="pssp", bufs=1, space="PSUM"))
        dram = ctx.enter_context(tc.tile_pool(name="dram", bufs=1, space="DRAM"))

        # ---- one-time loads ----
        ctx_sb = []
        for cc in range(4):
            t_ = singles.tile([128, T], bf16, tag=f"ctx{cc}")
            nc.sync.dma_start(out=t_, in_=ctxT[cc * 128:(cc + 1) * 128, :])
            ctx_sb.append(t_)
        pool_sb = []
        encb_sb = []
        for eh in range(2):
            p_ = singles.tile([128, POOL], bf16, tag=f"pool{eh}")
            nc.sync.dma_start(out=p_, in_=poolT[eh * 128:(eh + 1) * 128, :])
            pool_sb.append(p_)
            e_ = singles.tile([128, T], bf16, tag=f"encb{eh}")
            nc.sync.dma_start(out=e_, in_=encbT[eh * 128:(eh + 1) * 128, :])
            encb_sb.append(e_)

        ones_bf = singles.tile([128, 1], bf16, tag="ones_bf")
        nc.vector.memset(ones_bf, 1.0)
        ones_f32 = singles.tile([128, 1], f32, tag="ones_f32")
        nc.vector.memset(ones_f32, 1.0)
        outbuf = singles.tile([1, 2 * NPRED], f32, tag="outbuf")
        coll = singles.tile([128, 4 * NPRED], f32, tag="coll")
        nc.vector.memset(coll, 0.0)
        warm_rhs = singles.tile([128, 512], bf16, tag="warm_rhs")
        nc.vector.memset(warm_rhs, 0.0)
        biasq = singles.tile([128, 1], f32, tag="biasq")
        nc.vector.memset(biasq, float(math.log(1.0 / TEMP)))


        # ---- phase 1: all horizons' zh, norms, pos sums, and q columns ----
        zh_all = {}
        qcol_all = {}
        # warm the PE clock gate (HAM) with throwaway matmuls while the
        # input DMAs are in flight; ~4us of PE activity flips K to 8/8.
        wps = psum_zh.tile([128, 2, 255], f32, tag="zhps", name="warm_ps")
        for i in range(10):
            nc.tensor.matmul(wps[:1, 0, :255], ones_bf, warm_rhs[:, :255],
                             start=(i == 0), stop=(i == 9))

        def emit_p1(kk):
            Tk = T - 1 - kk  # 255 - kk rows on this core
            R = [128, Tk - 128]

            # W[k].T in one DMA: [c(4x128), e 256] -> [128, 4, 256]
            wtile = wt_pool.tile([128, 4, C_ENC], bf16, tag="wt")
            nc.gpsimd.dma_start(
                out=wtile,
                in_=WT[kk].rearrange("(cc p) e -> p cc e", p=128),
            )

            # matmul1: zhT[e, t] (2 e-halves in one psum bank)
            zps = psum_zh.tile([128, 2, Tk], f32, tag="zhps")
            for eh in range(2):
                for cc in range(4):
                    nc.tensor.matmul(
                        zps[:, eh, :],
                        wtile[:, cc, eh * 128:(eh + 1) * 128],
                        ctx_sb[cc][:, :Tk],
                        start=(cc == 0),
                        stop=(cc == 3),
                    )
            z_ = zh_pool.tile([128, 2, Tk], bf16, tag=f"zh{kk}", name=f"zh{kk}")
            half = Tk // 2
            nc.scalar.activation(
                out=z_[:, :, :half], in_=zps[:, :, :half], func=AF.Copy)
            nc.vector.tensor_copy(out=z_[:, :, half:], in_=zps[:, :, half:])
            zh_sb = [z_[:, 0, :], z_[:, 1, :]]
            zh_all[kk] = zh_sb

            # zh^2 and zh*pos packed into one [128, 2*Tk] tile per e-half
            bgs = []
            for eh in range(2):
                bg = big_pool.tile([128, 2 * Tk], bf16, tag=f"big{eh}")
                nc.vector.tensor_mul(bg[:, :Tk], zh_sb[eh], zh_sb[eh])
                nc.vector.tensor_mul(
                    bg[:, Tk:2 * Tk], zh_sb[eh], encb_sb[eh][:, kk + 1:kk + 1 + Tk]
                )
                bgs.append(bg)

            # per t-half: ss and pos column vectors via tiny ones-rhs matmuls
            # (bg as lhsT: out[t, 1] = sum_e bg[e, t]); then
            # q = 1/(sqrt(ss)*TEMP) = exp(-0.5*ln(ss) + ln(1/TEMP)) on columns.
            # both t-halves in one [128, 4] psum tile: cols = ss_h0, pos_h0,
            # ss_h1, pos_h1; strided Ln/Exp/mul handle both halves at once.
            sp4 = psum_sp.tile([128, 4], f32, tag="sp", name=f"sp{kk}")
            for h in range(2):
                Rh = R[h]
                for eh in range(2):
                    nc.tensor.matmul(
                        sp4[:Rh, 2 * h:2 * h + 1],
                        bgs[eh][:, h * 128:h * 128 + Rh],
                        ones_bf,
                        start=(eh == 0),
                        stop=(eh == 1),
                    )
                for eh in range(2):
                    nc.tensor.matmul(
                        sp4[:Rh, 2 * h + 1:2 * h + 2],
                        bgs[eh][:, Tk + h * 128:Tk + h * 128 + Rh],
                        ones_bf,
                        start=(eh == 0),
                        stop=(eh == 1),
                    )
            lnc = sv_pool.tile([128, 2], f32, tag="lnc")
            nc.scalar.activation(out=lnc, in_=sp4[:, 0:4:2], func=AF.Ln)
            qc = qcol_pool.tile([128, 2], f32, tag=f"qc{kk}", name=f"qc{kk}")
            nc.scalar.activation(
                out=qc, in_=lnc, func=AF.Exp,
                scale=-0.5, bias=biasq,
            )
            # pos~ columns straight into the collection tile (cols 24..47)
            nc.vector.tensor_mul(
                coll[:, 2 * NPRED + 2 * kk:2 * NPRED + 2 * kk + 2],
                sp4[:, 1:4:2],
                qc,
            )
            qcol_all[kk] = [qc[:128, 0:1], qc[:R[1], 1:2]]

        # ---- phase 2: full-pool logits, exp, masked reduce per horizon ----
        def emit_p2(kk):
            Tk = T - 1 - kk
            R = [128, Tk - 128]
            zh_sb = zh_all[kk]
            q_col = qcol_all[kk]

            for h in range(2):
                Rh = R[h]
                cb = cnt_pool.tile([128, POOL], DT.float8e4, tag="cnt")
                nc.sync.dma_start(
                    out=cb[:Rh, :],
                    in_=cnt[kk, h * 128:h * 128 + Rh, :],
                )
                ee = e_pool.tile([128, POOL], DT.float8e5, tag="E")
                for g in range(2):
                    lp = psum_L.tile([128, 2, 512], f32, tag="L", name=f"lp{g}")
                    for i in range(2):
                        mc = 2 * g + i
                        for eh in range(2):
                            nc.tensor.matmul(
                                lp[:Rh, i, :],
                                zh_sb[eh][:, h * 128:h * 128 + Rh],
                                pool_sb[eh][:, mc * 512:(mc + 1) * 512],
                                start=(eh == 0),
                                stop=(eh == 1),
                            )
                    nc.scalar.activation(
                        out=ee[:Rh, g * 1024:(g + 1) * 1024],
                        in_=lp[:Rh, :, :],
                        func=AF.Exp,
                        scale=q_col[h],
                    )
                to_ = tout_pool.tile([128, POOL], DT.float8e4, tag="to")
                s_cur = sv_pool.tile([128, 1], f32, tag="scur")
                nc.vector.affine_mul_reduce(
                    out=to_[:Rh, :],
                    accum_out=s_cur[:Rh, :],
                    in0=ee[:Rh, :],
                    in1=cb[:Rh, :],
                    scale=1.0,
                    bias=0.0,
                )
                j = 2 * kk + h
                nc.scalar.activation(
                    out=coll[:Rh, j:j + 1], in_=s_cur[:Rh, :], func=AF.Ln
                )

        for kk in range(NPRED):
            emit_p1(kk)
        for kk in range(NPRED):
            emit_p2(kk)

        fin = psum_sp.tile([1, 4 * NPRED], f32, tag="sp", name="fin")
        nc.tensor.matmul(fin, ones_f32, coll, start=True, stop=True)
        fin_sb = singles.tile([1, 4 * NPRED], f32, tag="fin_sb")
        nc.vector.tensor_copy(out=fin_sb, in_=fin)
        nc.vector.tensor_add(
            outbuf[:, 0:2 * NPRED],
            fin_sb[:, 0:4 * NPRED:2],
            fin_sb[:, 1:4 * NPRED:2],
        )
        nc.sync.dma_start(out=out_sums, in_=outbuf)


def _build_cnt(neg_idx):
    """cnt[b, kk, t, m] (uint8): negative multiplicities + 1 at the positive."""
    cnt = np.zeros((B, NPRED, T, POOL), dtype=np.uint8)
    b_ar = np.arange(B, dtype=np.int64)[:, None]
    for kk in range(NPRED):
        Tk = T - 1 - kk
        Nk = B * Tk
        idx = np.asarray(neg_idx[kk, :Nk], dtype=np.int64)  # [Nk, K]
        flat = idx + np.arange(Nk, dtype=np.int64)[:, None] * POOL
        c = np.bincount(flat.ravel(), minlength=Nk * POOL)
        c = c.astype(np.uint8).reshape(B, Tk, POOL)
        t_ar = np.arange(Tk, dtype=np.int64)[None, :]
        pos_m = T * b_ar + (kk + 1) + t_ar
        c[b_ar, t_ar, pos_m] += 1
        cnt[:, kk, :Tk, :] = c
    return cnt


def kernel(context, encoded, W, neg_idx):
    context = np.asarray(context, dtype=np.float32)
    encoded = np.asarray(encoded, dtype=np.float32)
    W = np.asarray(W, dtype=np.float32)
    neg_idx = np.asarray(neg_idx)

    if "nc" not in _CACHE:
        _CACHE["nc"] = _build_program()
    nc = _CACHE["nc"]

    # host prep: layout transposes, normalized pool, count mask
    pool = encoded.reshape(POOL, C_ENC)
    pool_n = pool / np.linalg.norm(pool, axis=-1, keepdims=True)
    poolT_bf = np.ascontiguousarray(pool_n.T).astype(BF16)  # [256, 2048]
    WT_bf = np.ascontiguousarray(W.transpose(0, 2, 1)).astype(BF16)  # [12, 512, 256]
    cnt_u8 = _build_cnt(neg_idx)
    fp8_lut = np.arange(256).astype(np.float32).astype(FP8)

    in_maps = []
    for b in range(N_CORES):
        in_maps.append(
            {
                "ctxT": np.ascontiguousarray(context[b].T).astype(BF16),
                "WT": WT_bf,
                "poolT": poolT_bf,
                "encbT": np.ascontiguousarray(poolT_bf[:, T * b:T * (b + 1)]),
                "cnt": fp8_lut[cnt_u8[b]],
            }
        )

    _CACHE["in_maps"] = in_maps
    res = bass_utils.run_bass_kernel_spmd(nc, in_maps, core_ids=list(range(N_CORES)))

    total = np.float64(0.0)
    for kk in range(NPRED):
        Tk = T - 1 - kk
        num = np.float64(0.0)
        for b in range(N_CORES):
            sums = res.results[b]["out_sums"][0]
            num += np.float64(sums[kk]) - np.float64(sums[NPRED + kk])
        total += num / (B * Tk)
    total = total / NPRED
    return np.float32(total)
